# revision 13
# baseline (speedup 1.0000x reference)
"""Multi-head self-attention (B=2, S=2048, D=1024, H=16) on 8 TRN2 NeuronCores.

Sharding: head-parallel — 2 heads per core. Each core computes Q/K/V
projections for its 2 heads over all B*S tokens, full (non-causal)
softmax attention for its 4 (batch, head) units, and a partial output
projection y_c = sum_h out_h @ wo[h]. Host sums the 8 partial outputs.

Device dataflow (everything "transposed": head-dim on partitions):
  x [T=4096, D=1024]  --PE transpose-->  xT [D, T]
  q2t/k2t/v2t [128=2*64, T] = w[:,2heads]^T @ xT      (PSUM accum over D)
  v2t --PE transpose--> vnat [k, d] (+ ones column -> denominator row)
  scoresT[k, q] = K Q^T  (contract d=64), exp on ACT (scale=1/8 folded in)
  poT[d+1, q] += vnat^T @ exp  accumulated over k tiles  (PSUM)
  out2t[:, q] = poT[0:64] * (1/poT[64]) broadcast (K=1 matmul + DVE mul)
  y[s, n] = out2t[:, s-tile]^T @ wo2   (contract 128 = 2 heads * 64)

Matmuls run as float32r (tfloat32 datapath, full PE rate); every operand
tile is produced with an fp32r-rounding instruction as the BIR verifier
requires. PSUM accumulation stays fp32.
"""

import numpy as np
from contextlib import ExitStack

import concourse.bass as bass
import concourse.tile as tile
from concourse import bacc, mybir
from concourse.bass_utils import run_bass_kernel_spmd
from concourse.masks import make_identity

F32 = mybir.dt.float32
F32R = mybir.dt.float32r
AF = mybir.ActivationFunctionType

N_CORES = 8
D_MODEL = 1024
NUM_HEADS = 16
DEPTH = 64
HEADS_PER_CORE = NUM_HEADS // N_CORES  # 2
B_FULL = 2
S_FULL = 2048


def build_program(T=4096, D=1024, S=2048, dh=64, hc=2, with_qkv_bias=False,
                  with_o_bias=False, use_f32r=True):
    """Build the SPMD Bass program for one core (hc heads).

    T: total tokens (B*S); D: model dim; S: seq len per batch; dh: head depth;
    hc: heads handled by this core. Requires hc*dh == 128, D % 512 == 0,
    S % 512 == 0, T % S == 0.
    """
    d2 = hc * dh
    assert d2 == 128 and D % 128 == 0 and S % 512 == 0 and T % S == 0
    nb = T // S            # batches
    ndc = D // 128         # D chunks (contraction tiles)
    nch = T // 512         # 512-token chunks
    KT = S // 128          # k tiles per (b,h) unit
    QC = S // 512          # 512-wide q chunks per batch
    scale = 1.0 / float(np.sqrt(dh))
    MDT = F32R if use_f32r else F32  # dtype of tiles feeding matmuls

    nc = bacc.Bacc("TRN2", target_bir_lowering=False, debug=False,
                   num_devices=N_CORES)

    x_d = nc.dram_tensor("x", [T, D], F32, kind="ExternalInput").ap()
    wq_d = nc.dram_tensor("wq", [D, d2], F32, kind="ExternalInput").ap()
    wk_d = nc.dram_tensor("wk", [D, d2], F32, kind="ExternalInput").ap()
    wv_d = nc.dram_tensor("wv", [D, d2], F32, kind="ExternalInput").ap()
    wo_d = nc.dram_tensor("wo", [d2, D], F32, kind="ExternalInput").ap()
    if with_qkv_bias:
        bq_d = nc.dram_tensor("bq", [d2, 1], F32, kind="ExternalInput").ap()
        bk_d = nc.dram_tensor("bk", [d2, 1], F32, kind="ExternalInput").ap()
        bv_d = nc.dram_tensor("bv", [d2, 1], F32, kind="ExternalInput").ap()
    if with_o_bias:
        bo_d = nc.dram_tensor("bo", [1, D], F32, kind="ExternalInput").ap()
    y_d = nc.dram_tensor("y", [T, D], F32, kind="ExternalOutput").ap()

    with tile.TileContext(nc) as tc, ExitStack() as ctx:
        singles = ctx.enter_context(tc.tile_pool(name="singles", bufs=1))
        xpool = ctx.enter_context(tc.tile_pool(name="xpool", bufs=3))
        xtpool = ctx.enter_context(tc.tile_pool(name="xtpool", bufs=2))
        v2pool = ctx.enter_context(tc.tile_pool(name="v2pool", bufs=2))
        epool = ctx.enter_context(tc.tile_pool(name="epool", bufs=3))
        ysb = ctx.enter_context(tc.tile_pool(name="ysb", bufs=3))
        rcpool = ctx.enter_context(tc.tile_pool(name="rcpool", bufs=2))
        psA = ctx.enter_context(tc.tile_pool(name="psA", bufs=3, space="PSUM"))
        psO = ctx.enter_context(tc.tile_pool(name="psO", bufs=4, space="PSUM"))

        ident = singles.tile([128, 128], F32)
        make_identity(nc, ident[:])
        # memset can't write fp32r; memset fp32 then round via copy
        ones1f = singles.tile([1, 64], F32)
        nc.vector.memset(ones1f[:], 1.0)
        ones1 = singles.tile([1, 64], MDT)
        nc.vector.tensor_copy(ones1[:], ones1f[:])

        # weights: load fp32, then round on-chip to the matmul dtype.
        # layout [128, ndc, 128]: contraction chunk dc on the middle dim
        w_sb = []
        with tc.tile_pool(name="wraw", bufs=2) as wraw:
            for name, wd in (("wqs", wq_d), ("wks", wk_d), ("wvs", wv_d)):
                raw = wraw.tile([128, ndc, d2], F32, tag="wr", name=f"raw_{name}")
                nc.sync.dma_start(out=raw[:],
                                  in_=wd.rearrange("(dc p) m -> p dc m", p=128))
                t = singles.tile([128, ndc, d2], MDT, tag=name, name=name)
                nc.vector.tensor_copy(t[:], raw[:])
                w_sb.append(t)
            raw = wraw.tile([d2, D], F32, tag="wr", name="raw_wo")
            nc.sync.dma_start(out=raw[:], in_=wo_d)
            wo_sb = singles.tile([d2, D], MDT)
            nc.vector.tensor_copy(wo_sb[:], raw[:])

        b_sb = [None, None, None]
        if with_qkv_bias:
            for i, bd in enumerate((bq_d, bk_d, bv_d)):
                t = singles.tile([d2, 1], F32, tag=f"b{i}", name=f"b{i}")
                nc.sync.dma_start(out=t[:], in_=bd)
                b_sb[i] = t
        bo_sb = None
        if with_o_bias:
            # DMA-broadcast bo across all 128 partitions (DVE can't
            # partition-broadcast, so materialize it once)
            bo_sb = singles.tile([128, D], F32)
            nc.gpsimd.dma_start(out=bo_sb[:], in_=bo_d.partition_broadcast(128))

        q2t = singles.tile([128, T], MDT, tag="q2t")
        k2t = singles.tile([128, T], MDT, tag="k2t")
        out2t = singles.tile([128, T], MDT, tag="out2t")
        # vnat[:, u, kt, 0:64] = V rows (k on partitions); col 64 = ones
        vnat = singles.tile([128, nb * hc, KT, dh + 1], MDT, tag="vnat")
        onesc = singles.tile([128, nb * hc, KT, 1], F32)
        nc.vector.memset(onesc[:], 1.0)
        nc.vector.tensor_copy(vnat[:, :, :, dh:dh + 1], onesc[:])

        # ---- phase 1+2: transpose x and project, one 512-token chunk at a time
        for n in range(nch):
            xt_n = xtpool.tile([128, ndc, 512], MDT, tag="xt")
            for st in range(4):  # 128-token s-tiles within the chunk
                tok0 = n * 512 + st * 128
                xx = xpool.tile([128, D], F32, tag="xx")
                nc.sync.dma_start(out=xx[:], in_=x_d[tok0:tok0 + 128, :])
                for dc0 in range(0, ndc, 4):
                    g = min(4, ndc - dc0)
                    pt = psA.tile([128, g, 128], F32, tag="big")
                    for i in range(g):
                        nc.tensor.transpose(
                            pt[:, i, :], xx[:, (dc0 + i) * 128:(dc0 + i + 1) * 128],
                            ident[:])
                    nc.vector.tensor_copy(
                        xt_n[:, dc0:dc0 + g, st * 128:(st + 1) * 128], pt[:])

            for p in range(3):
                ps = psA.tile([128, 512], F32, tag="big")
                for dc in range(ndc):
                    nc.tensor.matmul(ps[:], w_sb[p][:, dc, :], xt_n[:, dc, :],
                                     start=(dc == 0), stop=(dc == ndc - 1))
                if p < 2:
                    dst = (q2t, k2t)[p]
                    if with_qkv_bias:
                        nc.vector.tensor_scalar_add(
                            dst[:, n * 512:(n + 1) * 512], ps[:], b_sb[p][:])
                    else:
                        nc.vector.tensor_copy(dst[:, n * 512:(n + 1) * 512], ps[:])
                else:
                    v2_n = v2pool.tile([128, 512], F32, tag="v2")
                    if with_qkv_bias:
                        nc.vector.tensor_scalar_add(v2_n[:], ps[:], b_sb[2][:])
                    else:
                        nc.vector.tensor_copy(v2_n[:], ps[:])
                    # transpose V chunk into natural [k, d] layout; one
                    # [128,128] transpose covers both heads (cols split after)
                    b = (n * 512) // S
                    kt0 = (n * 512 % S) // 128
                    pv = psA.tile([128, 4, 128], F32, tag="big")
                    for sub in range(4):
                        nc.tensor.transpose(
                            pv[:, sub, :],
                            v2_n[:, sub * 128:(sub + 1) * 128], ident[:])
                    for h in range(hc):
                        nc.vector.tensor_copy(
                            vnat[:, b * hc + h, kt0:kt0 + 4, 0:dh],
                            pv[:, :, h * dh:(h + 1) * dh])

        # ---- phase 3: attention per (batch, head) unit
        for b in range(nb):
            for h in range(hc):
                u = b * hc + h
                hp = slice(h * dh, (h + 1) * dh)
                po = [psO.tile([dh + 1, 512], F32, tag="po", name=f"po{u}_{i}")
                      for i in range(QC)]
                for kt in range(KT):
                    kcol = slice(b * S + kt * 128, b * S + (kt + 1) * 128)
                    for qc in range(QC):
                        qcol = slice(b * S + qc * 512, b * S + (qc + 1) * 512)
                        sc = psA.tile([128, 512], F32, tag="big")
                        nc.tensor.matmul(sc[:], k2t[hp, kcol], q2t[hp, qcol],
                                         start=True, stop=True)
                        ex = epool.tile([128, 512], MDT, tag="ex")
                        nc.scalar.activation(ex[:], sc[:], AF.Exp, scale=scale)
                        nc.tensor.matmul(po[qc][:], vnat[:, u, kt, :], ex[:],
                                         start=(kt == 0), stop=(kt == KT - 1))
                for qc in range(QC):
                    qcol = slice(b * S + qc * 512, b * S + (qc + 1) * 512)
                    rc = rcpool.tile([1, 512], MDT, tag="rc")
                    with nc.allow_low_precision(reason="softmax denom, tf32 ok"):
                        nc.vector.reciprocal(rc[:], po[qc][dh:dh + 1, :])
                    # broadcast 1/denom across dh partitions via K=1 matmul
                    rcp = psA.tile([dh, 512], F32, tag="big")
                    nc.tensor.matmul(rcp[:], ones1[:], rc[:],
                                     start=True, stop=True)
                    rcb = rcpool.tile([dh, 512], F32, tag="rcb")
                    nc.scalar.activation(rcb[:], rcp[:], AF.Copy)
                    nc.vector.tensor_mul(out2t[hp, qcol], po[qc][0:dh, :], rcb[:])

        # ---- phase 4: partial output projection
        NJ = min(512, D)
        for m in range(T // 128):
            for j in range(D // NJ):
                py = psA.tile([128, NJ], F32, tag="big")
                nc.tensor.matmul(py[:], out2t[:, m * 128:(m + 1) * 128],
                                 wo_sb[:, j * NJ:(j + 1) * NJ],
                                 start=True, stop=True)
                yt = ysb.tile([128, NJ], F32, tag="yt")
                if with_o_bias:
                    nc.vector.tensor_add(yt[:], py[:],
                                         bo_sb[:, j * NJ:(j + 1) * NJ])
                else:
                    nc.vector.tensor_copy(yt[:], py[:])
                nc.sync.dma_start(out=y_d[m * 128:(m + 1) * 128,
                                          j * NJ:(j + 1) * NJ], in_=yt[:])

    nc.compile()
    return nc


_PROGRAM_CACHE = {}


def _get_program(key):
    if key not in _PROGRAM_CACHE:
        with_qkv_bias, with_o_bias = key
        _PROGRAM_CACHE[key] = build_program(
            with_qkv_bias=with_qkv_bias, with_o_bias=with_o_bias)
    return _PROGRAM_CACHE[key]


def make_in_maps(x, wq, bq, wk, bk, wv, bv, wo, bo, with_qkv_bias, with_o_bias,
                 n_cores=N_CORES, hc=HEADS_PER_CORE, dh=DEPTH):
    d2 = hc * dh
    in_maps = []
    for c in range(n_cores):
        cs = slice(c * d2, (c + 1) * d2)
        m = {"x": x,
             "wq": np.ascontiguousarray(wq[:, cs]),
             "wk": np.ascontiguousarray(wk[:, cs]),
             "wv": np.ascontiguousarray(wv[:, cs]),
             "wo": np.ascontiguousarray(wo[cs, :])}
        if with_qkv_bias:
            m["bq"] = np.ascontiguousarray(bq[cs].reshape(d2, 1))
            m["bk"] = np.ascontiguousarray(bk[cs].reshape(d2, 1))
            m["bv"] = np.ascontiguousarray(bv[cs].reshape(d2, 1))
        if with_o_bias:
            m["bo"] = (bo.reshape(1, -1).astype(np.float32) if c == 0
                       else np.zeros((1, bo.shape[-1]), np.float32))
        in_maps.append(m)
    return in_maps


def kernel(inputs, wq, bq, wk, bk, wv, bv, wo, bo):
    x = np.ascontiguousarray(np.asarray(inputs, np.float32)
                             .reshape(B_FULL * S_FULL, D_MODEL))
    wq, wk, wv, wo = (np.asarray(a, np.float32) for a in (wq, wk, wv, wo))
    bq, bk, bv, bo = (np.asarray(a, np.float32) for a in (bq, bk, bv, bo))

    with_qkv_bias = bool(np.any(bq) or np.any(bk) or np.any(bv))
    with_o_bias = bool(np.any(bo))
    nc = _get_program((with_qkv_bias, with_o_bias))

    in_maps = make_in_maps(x, wq, bq, wk, bk, wv, bv, wo, bo,
                           with_qkv_bias, with_o_bias)
    res = run_bass_kernel_spmd(nc, in_maps, list(range(N_CORES))).results
    y = np.zeros((B_FULL * S_FULL, D_MODEL), np.float64)
    for c in range(N_CORES):
        y += res[c]["y"]
    return y.astype(np.float32).reshape(B_FULL, S_FULL, D_MODEL)


# revision 16
# speedup vs baseline: 1.0881x; 1.0881x over previous
"""Multi-head self-attention (B=2, S=2048, D=1024, H=16) on 8 TRN2 NeuronCores.

Sharding: head-parallel — 2 heads per core. Each core computes Q/K/V
projections for its 2 heads over all B*S tokens, full (non-causal)
softmax attention for its 4 (batch, head) units, and a partial output
projection y_c = sum_h out_h @ wo[h]. Host sums the 8 partial outputs.

Device dataflow (everything "transposed": head-dim on partitions):
  x [T=4096, D=1024]  --PE transpose-->  xT [D, T]
  q2t/k2t/v2t [128=2*64, T] = w[:,2heads]^T @ xT      (PSUM accum over D)
  v2t --PE transpose--> vnat [k, d] (+ ones column -> denominator row)
  scoresT[k, q] = K Q^T  (contract d=64), exp on ACT (scale=1/8 folded in)
  poT[d+1, q] += vnat^T @ exp  accumulated over k tiles  (PSUM)
  out2t[:, q] = poT[0:64] * (1/poT[64]) broadcast (K=1 matmul + DVE mul)
  y[s, n] = out2t[:, s-tile]^T @ wo2   (contract 128 = 2 heads * 64)

Matmuls run as float32r (tfloat32 datapath, full PE rate); every operand
tile is produced with an fp32r-rounding instruction as the BIR verifier
requires. PSUM accumulation stays fp32.
"""

import numpy as np
from contextlib import ExitStack

import concourse.bass as bass
import concourse.tile as tile
from concourse import bacc, mybir
from concourse.bass_utils import run_bass_kernel_spmd
from concourse.masks import make_identity

F32 = mybir.dt.float32
F32R = mybir.dt.float32r
AF = mybir.ActivationFunctionType

N_CORES = 8
D_MODEL = 1024
NUM_HEADS = 16
DEPTH = 64
HEADS_PER_CORE = NUM_HEADS // N_CORES  # 2
B_FULL = 2
S_FULL = 2048


def build_program(T=4096, D=1024, S=2048, dh=64, hc=2, with_qkv_bias=False,
                  with_o_bias=False, use_f32r=True):
    """Build the SPMD Bass program for one core (hc heads).

    T: total tokens (B*S); D: model dim; S: seq len per batch; dh: head depth;
    hc: heads handled by this core. Requires hc*dh == 128, D % 512 == 0,
    S % 512 == 0, T % S == 0.
    """
    d2 = hc * dh
    assert d2 == 128 and D % 128 == 0 and S % 512 == 0 and T % S == 0
    nb = T // S            # batches
    ndc = D // 128         # D chunks (contraction tiles)
    nch = T // 512         # 512-token chunks
    KT = S // 128          # k tiles per (b,h) unit
    QC = S // 512          # 512-wide q chunks per batch
    scale = 1.0 / float(np.sqrt(dh))
    MDT = F32R if use_f32r else F32  # dtype of tiles feeding matmuls

    nc = bacc.Bacc("TRN2", target_bir_lowering=False, debug=False,
                   num_devices=N_CORES)

    x_d = nc.dram_tensor("x", [T, D], F32, kind="ExternalInput").ap()
    wq_d = nc.dram_tensor("wq", [D, d2], F32, kind="ExternalInput").ap()
    wk_d = nc.dram_tensor("wk", [D, d2], F32, kind="ExternalInput").ap()
    wv_d = nc.dram_tensor("wv", [D, d2], F32, kind="ExternalInput").ap()
    wo_d = nc.dram_tensor("wo", [d2, D], F32, kind="ExternalInput").ap()
    if with_qkv_bias:
        bq_d = nc.dram_tensor("bq", [d2, 1], F32, kind="ExternalInput").ap()
        bk_d = nc.dram_tensor("bk", [d2, 1], F32, kind="ExternalInput").ap()
        bv_d = nc.dram_tensor("bv", [d2, 1], F32, kind="ExternalInput").ap()
    if with_o_bias:
        bo_d = nc.dram_tensor("bo", [1, D], F32, kind="ExternalInput").ap()
    y_d = nc.dram_tensor("y", [T, D], F32, kind="ExternalOutput").ap()

    with tile.TileContext(nc) as tc, ExitStack() as ctx:
        singles = ctx.enter_context(tc.tile_pool(name="singles", bufs=1))
        xpool = ctx.enter_context(tc.tile_pool(name="xpool", bufs=3))
        xtpool = ctx.enter_context(tc.tile_pool(name="xtpool", bufs=2))
        v2pool = ctx.enter_context(tc.tile_pool(name="v2pool", bufs=2))
        epool = ctx.enter_context(tc.tile_pool(name="epool", bufs=3))
        ysb = ctx.enter_context(tc.tile_pool(name="ysb", bufs=3))
        rcpool = ctx.enter_context(tc.tile_pool(name="rcpool", bufs=2))

        ident = singles.tile([128, 128], F32)
        make_identity(nc, ident[:])
        # memset can't write fp32r; memset fp32 then round via copy
        ones1f = singles.tile([1, 64], F32)
        nc.vector.memset(ones1f[:], 1.0)
        ones1 = singles.tile([1, 64], MDT)
        nc.vector.tensor_copy(ones1[:], ones1f[:])

        # weights: load fp32, then round on-chip to the matmul dtype.
        # layout [128, ndc, 128]: contraction chunk dc on the middle dim
        w_sb = []
        with tc.tile_pool(name="wraw", bufs=2) as wraw:
            for name, wd in (("wqs", wq_d), ("wks", wk_d), ("wvs", wv_d)):
                raw = wraw.tile([128, ndc, d2], F32, tag="wr", name=f"raw_{name}")
                nc.sync.dma_start(out=raw[:],
                                  in_=wd.rearrange("(dc p) m -> p dc m", p=128))
                t = singles.tile([128, ndc, d2], MDT, tag=name, name=name)
                nc.vector.tensor_copy(t[:], raw[:])
                w_sb.append(t)
            raw = wraw.tile([d2, D], F32, tag="wr", name="raw_wo")
            nc.sync.dma_start(out=raw[:], in_=wo_d)
            wo_sb = singles.tile([d2, D], MDT)
            nc.vector.tensor_copy(wo_sb[:], raw[:])

        b_sb = [None, None, None]
        if with_qkv_bias:
            for i, bd in enumerate((bq_d, bk_d, bv_d)):
                t = singles.tile([d2, 1], F32, tag=f"b{i}", name=f"b{i}")
                nc.sync.dma_start(out=t[:], in_=bd)
                b_sb[i] = t
        bo_sb = None
        if with_o_bias:
            # DMA-broadcast bo across all 128 partitions (DVE can't
            # partition-broadcast, so materialize it once)
            bo_sb = singles.tile([128, D], F32)
            nc.gpsimd.dma_start(out=bo_sb[:], in_=bo_d.partition_broadcast(128))

        q2t = singles.tile([128, T], MDT, tag="q2t")
        k2t = singles.tile([128, T], MDT, tag="k2t")
        out2t = singles.tile([128, T], MDT, tag="out2t")
        # vnat[:, u, kt, 0:64] = V rows (k on partitions); col 64 = ones
        vnat = singles.tile([128, nb * hc, KT, dh + 1], MDT, tag="vnat")
        onesc = singles.tile([128, nb * hc, KT, 1], F32)
        nc.vector.memset(onesc[:], 1.0)
        nc.vector.tensor_copy(vnat[:, :, :, dh:dh + 1], onesc[:])

        # ---- phase 1+2: transpose x and project, one 512-token chunk at a time
        with tc.tile_pool(name="ps12", bufs=6, space="PSUM") as ps12:
            for n in range(nch):
                xt_n = xtpool.tile([128, ndc, 512], MDT, tag="xt")
                for st in range(4):  # 128-token s-tiles within the chunk
                    tok0 = n * 512 + st * 128
                    xx = xpool.tile([128, D], F32, tag="xx")
                    nc.sync.dma_start(out=xx[:], in_=x_d[tok0:tok0 + 128, :])
                    for dc0 in range(0, ndc, 4):
                        g = min(4, ndc - dc0)
                        pt = ps12.tile([128, g, 128], F32, tag="b12")
                        for i in range(g):
                            nc.tensor.transpose(
                                pt[:, i, :],
                                xx[:, (dc0 + i) * 128:(dc0 + i + 1) * 128],
                                ident[:])
                        nc.vector.tensor_copy(
                            xt_n[:, dc0:dc0 + g, st * 128:(st + 1) * 128], pt[:])

                for p in range(3):
                    ps = ps12.tile([128, 512], F32, tag="b12")
                    for dc in range(ndc):
                        nc.tensor.matmul(ps[:], w_sb[p][:, dc, :], xt_n[:, dc, :],
                                         start=(dc == 0), stop=(dc == ndc - 1))
                    if p < 2:
                        dst = (q2t, k2t)[p]
                        if with_qkv_bias:
                            nc.vector.tensor_scalar_add(
                                dst[:, n * 512:(n + 1) * 512], ps[:], b_sb[p][:])
                        else:
                            nc.vector.tensor_copy(
                                dst[:, n * 512:(n + 1) * 512], ps[:])
                    else:
                        v2_n = v2pool.tile([128, 512], F32, tag="v2")
                        if with_qkv_bias:
                            nc.vector.tensor_scalar_add(v2_n[:], ps[:], b_sb[2][:])
                        else:
                            nc.vector.tensor_copy(v2_n[:], ps[:])
                        # transpose V chunk into natural [k, d] layout; one
                        # [128,128] transpose covers both heads
                        b = (n * 512) // S
                        kt0 = (n * 512 % S) // 128
                        pv = ps12.tile([128, 4, 128], F32, tag="b12")
                        for sub in range(4):
                            nc.tensor.transpose(
                                pv[:, sub, :],
                                v2_n[:, sub * 128:(sub + 1) * 128], ident[:])
                        for h in range(hc):
                            nc.vector.tensor_copy(
                                vnat[:, b * hc + h, kt0:kt0 + 4, 0:dh],
                                pv[:, :, h * dh:(h + 1) * dh])

        # ---- phase 3: attention per (batch, head) unit, software-pipelined:
        # po(kt-1) is issued after sc(kt) so the PE never blocks on the exp
        # of the same k-tile it just produced.
        with tc.tile_pool(name="psS", bufs=2, space="PSUM") as psS, \
             tc.tile_pool(name="psO", bufs=4, space="PSUM") as psO:
            NE = (QC + 1) // 2  # exp groups of 2 q-chunks
            for b in range(nb):
                for h in range(hc):
                    u = b * hc + h
                    hp = slice(h * dh, (h + 1) * dh)
                    po = [psO.tile([dh + 1, 512], F32, tag="po",
                                   name=f"po{u}_{i}") for i in range(QC)]

                    def issue_po(kt, exs):
                        for qc in range(QC):
                            nc.tensor.matmul(po[qc][:], vnat[:, u, kt, :],
                                             exs[qc],
                                             start=(kt == 0),
                                             stop=(kt == KT - 1))

                    prev_exs = None
                    for kt in range(KT):
                        kcol = slice(b * S + kt * 128, b * S + (kt + 1) * 128)
                        scs = []
                        for e in range(NE):
                            g = min(2, QC - e * 2)
                            sc2 = psS.tile([128, g, 512], F32, tag="sc",
                                           name=f"sc{u}_{kt}_{e}")
                            for i in range(g):
                                qc = e * 2 + i
                                qcol = slice(b * S + qc * 512,
                                             b * S + (qc + 1) * 512)
                                nc.tensor.matmul(sc2[:, i, :], k2t[hp, kcol],
                                                 q2t[hp, qcol],
                                                 start=True, stop=True)
                            scs.append((sc2, g))
                        exs = [None] * QC
                        for e, (sc2, g) in enumerate(scs):
                            ex2 = epool.tile([128, g, 512], MDT, tag="ex",
                                             name=f"ex{u}_{kt}_{e}")
                            nc.scalar.activation(ex2[:], sc2[:], AF.Exp,
                                                 scale=scale)
                            for i in range(g):
                                exs[e * 2 + i] = ex2[:, i, :]
                        if kt > 0:
                            issue_po(kt - 1, prev_exs)
                        prev_exs = exs
                    issue_po(KT - 1, prev_exs)

                    for qc in range(QC):
                        qcol = slice(b * S + qc * 512, b * S + (qc + 1) * 512)
                        rc = rcpool.tile([1, 512], MDT, tag="rc")
                        with nc.allow_low_precision(reason="softmax denom"):
                            nc.vector.reciprocal(rc[:], po[qc][dh:dh + 1, :])
                        # broadcast 1/denom across dh partitions via K=1 matmul
                        rcp = psS.tile([dh, 512], F32, tag="sc",
                                       name=f"rcp{u}_{qc}")
                        nc.tensor.matmul(rcp[:], ones1[:], rc[:],
                                         start=True, stop=True)
                        rcb = rcpool.tile([dh, 512], F32, tag="rcb")
                        nc.scalar.activation(rcb[:], rcp[:], AF.Copy)
                        nc.vector.tensor_mul(out2t[hp, qcol], po[qc][0:dh, :],
                                             rcb[:])

        # ---- phase 4: partial output projection
        NJ = min(512, D)
        with tc.tile_pool(name="psY", bufs=4, space="PSUM") as psY:
            for m in range(T // 128):
                for j in range(D // NJ):
                    py = psY.tile([128, NJ], F32, tag="py")
                    nc.tensor.matmul(py[:], out2t[:, m * 128:(m + 1) * 128],
                                     wo_sb[:, j * NJ:(j + 1) * NJ],
                                     start=True, stop=True)
                    yt = ysb.tile([128, NJ], F32, tag="yt")
                    if with_o_bias:
                        nc.vector.tensor_add(yt[:], py[:],
                                             bo_sb[:, j * NJ:(j + 1) * NJ])
                    else:
                        nc.vector.tensor_copy(yt[:], py[:])
                    nc.sync.dma_start(out=y_d[m * 128:(m + 1) * 128,
                                              j * NJ:(j + 1) * NJ], in_=yt[:])

    nc.compile()
    return nc


_PROGRAM_CACHE = {}


def _get_program(key):
    if key not in _PROGRAM_CACHE:
        with_qkv_bias, with_o_bias = key
        _PROGRAM_CACHE[key] = build_program(
            with_qkv_bias=with_qkv_bias, with_o_bias=with_o_bias)
    return _PROGRAM_CACHE[key]


def make_in_maps(x, wq, bq, wk, bk, wv, bv, wo, bo, with_qkv_bias, with_o_bias,
                 n_cores=N_CORES, hc=HEADS_PER_CORE, dh=DEPTH):
    d2 = hc * dh
    in_maps = []
    for c in range(n_cores):
        cs = slice(c * d2, (c + 1) * d2)
        m = {"x": x,
             "wq": np.ascontiguousarray(wq[:, cs]),
             "wk": np.ascontiguousarray(wk[:, cs]),
             "wv": np.ascontiguousarray(wv[:, cs]),
             "wo": np.ascontiguousarray(wo[cs, :])}
        if with_qkv_bias:
            m["bq"] = np.ascontiguousarray(bq[cs].reshape(d2, 1))
            m["bk"] = np.ascontiguousarray(bk[cs].reshape(d2, 1))
            m["bv"] = np.ascontiguousarray(bv[cs].reshape(d2, 1))
        if with_o_bias:
            m["bo"] = (bo.reshape(1, -1).astype(np.float32) if c == 0
                       else np.zeros((1, bo.shape[-1]), np.float32))
        in_maps.append(m)
    return in_maps


def kernel(inputs, wq, bq, wk, bk, wv, bv, wo, bo):
    x = np.ascontiguousarray(np.asarray(inputs, np.float32)
                             .reshape(B_FULL * S_FULL, D_MODEL))
    wq, wk, wv, wo = (np.asarray(a, np.float32) for a in (wq, wk, wv, wo))
    bq, bk, bv, bo = (np.asarray(a, np.float32) for a in (bq, bk, bv, bo))

    with_qkv_bias = bool(np.any(bq) or np.any(bk) or np.any(bv))
    with_o_bias = bool(np.any(bo))
    nc = _get_program((with_qkv_bias, with_o_bias))

    in_maps = make_in_maps(x, wq, bq, wk, bk, wv, bv, wo, bo,
                           with_qkv_bias, with_o_bias)
    res = run_bass_kernel_spmd(nc, in_maps, list(range(N_CORES))).results
    y = np.zeros((B_FULL * S_FULL, D_MODEL), np.float64)
    for c in range(N_CORES):
        y += res[c]["y"]
    return y.astype(np.float32).reshape(B_FULL, S_FULL, D_MODEL)


# revision 21
# speedup vs baseline: 1.5424x; 1.4175x over previous
"""Multi-head self-attention (B=2, S=2048, D=1024, H=16) on 8 TRN2 NeuronCores.

Sharding: head-parallel — 2 heads per core. Each core computes Q/K/V
projections for its 2 heads over all B*S tokens, full (non-causal)
softmax attention for its 4 (batch, head) units, and a partial output
projection y_c = sum_h out_h @ wo[h]. Host sums the 8 partial outputs.
The host also pre-transposes x to xT (pure layout prep) so the device
reads the contraction dim on partitions directly.

Device dataflow (head-dim on partitions):
  q2t/k2t/v2t [128=2*64, T] = w[:,2heads]^T @ xT      (PSUM accum over D)
  v2t --PE transpose--> vnat [k, d] (+ ones column -> denominator row)
  scoresT[k, q] = K Q^T  (contract d=64), exp on ACT (scale=1/8 folded in)
  poT[d+1, q] += vnat^T @ exp  accumulated over k tiles  (PSUM)
  out2t[:, q] = poT[0:64] * (1/poT[64]) broadcast (K=1 matmul + DVE mul)
  y[s, n] = out2t[:, s-tile]^T @ wo2   (contract 128 = 2 heads * 64)

Matmuls run as float32r (tfloat32 datapath, 1 cycle/row); every operand
tile is produced with an fp32r-rounding instruction as the BIR verifier
requires. PSUM accumulation stays fp32.

Emission is phase-interleaved so the PE always has independent filler
work while the ACT engine grinds through the exps: projections for
batch 1 ride along with batch 0's attention, and batch 0's output
projection rides along with batch 1's attention.
"""

import numpy as np
from contextlib import ExitStack

import concourse.bass as bass
import concourse.tile as tile
from concourse import bacc, mybir
from concourse.bass_utils import run_bass_kernel_spmd
from concourse.masks import make_identity

F32 = mybir.dt.float32
F32R = mybir.dt.float32r
AF = mybir.ActivationFunctionType

N_CORES = 8
D_MODEL = 1024
NUM_HEADS = 16
DEPTH = 64
HEADS_PER_CORE = NUM_HEADS // N_CORES  # 2
B_FULL = 2
S_FULL = 2048


def build_program(T=4096, D=1024, S=2048, dh=64, hc=2, with_qkv_bias=False,
                  with_o_bias=False, use_f32r=True, dma_f32r=True):
    """Build the SPMD Bass program for one core (hc heads).

    T: total tokens (B*S); D: model dim; S: seq len per batch; dh: head depth;
    hc: heads per core. Requires hc*dh == 128, D % 128 == 0, S % 512 == 0,
    T % S == 0. dma_f32r: DMA x directly into fp32r tiles (if the verifier
    allows DMA producers); else DMA to fp32 and round via DVE copy.
    """
    d2 = hc * dh
    assert d2 == 128 and D % 128 == 0 and S % 512 == 0 and T % S == 0
    nb = T // S            # batches
    ndc = D // 128         # D chunks (contraction tiles)
    cpb = S // 512         # 512-token chunks per batch
    KT = S // 128          # k tiles per (b,h) unit
    QC = S // 512          # 512-wide q chunks per batch
    NJ = min(512, D)
    scale = 1.0 / float(np.sqrt(dh))
    MDT = F32R if use_f32r else F32

    nc = bacc.Bacc("TRN2", target_bir_lowering=False, debug=False,
                   num_devices=N_CORES)

    xt_d = nc.dram_tensor("xt", [D, T], F32R if dma_f32r else F32,
                          kind="ExternalInput").ap()
    wq_d = nc.dram_tensor("wq", [D, d2], F32, kind="ExternalInput").ap()
    wk_d = nc.dram_tensor("wk", [D, d2], F32, kind="ExternalInput").ap()
    wv_d = nc.dram_tensor("wv", [D, d2], F32, kind="ExternalInput").ap()
    wo_d = nc.dram_tensor("wo", [d2, D], F32, kind="ExternalInput").ap()
    if with_qkv_bias:
        bq_d = nc.dram_tensor("bq", [d2, 1], F32, kind="ExternalInput").ap()
        bk_d = nc.dram_tensor("bk", [d2, 1], F32, kind="ExternalInput").ap()
        bv_d = nc.dram_tensor("bv", [d2, 1], F32, kind="ExternalInput").ap()
    if with_o_bias:
        bo_d = nc.dram_tensor("bo", [1, D], F32, kind="ExternalInput").ap()
    y_d = nc.dram_tensor("y", [T, D], F32, kind="ExternalOutput").ap()

    xt_view = xt_d.rearrange("(dc p) t -> p dc t", p=128)

    with tile.TileContext(nc) as tc, ExitStack() as ctx:
        singles = ctx.enter_context(tc.tile_pool(name="singles", bufs=1))
        xtpool = ctx.enter_context(tc.tile_pool(name="xtpool", bufs=3))
        v2pool = ctx.enter_context(tc.tile_pool(name="v2pool", bufs=2))
        epool = ctx.enter_context(tc.tile_pool(name="epool", bufs=4))
        ysb = ctx.enter_context(tc.tile_pool(name="ysb", bufs=3))
        rcpool = ctx.enter_context(tc.tile_pool(name="rcpool", bufs=2))
        # PSUM budget (8 banks): sc 2x[128,2,512]=4, ps 2x[128,512]=2,
        # po 2x[65,512]=2
        pspool = ctx.enter_context(tc.tile_pool(name="ps", bufs=2, space="PSUM"))
        psO = ctx.enter_context(tc.tile_pool(name="psO", bufs=2, space="PSUM"))
        posb = ctx.enter_context(tc.tile_pool(name="posb", bufs=2))

        ident = singles.tile([128, 128], F32)
        make_identity(nc, ident[:])
        ones1f = singles.tile([1, dh], F32)
        nc.vector.memset(ones1f[:], 1.0)
        ones1 = singles.tile([1, dh], MDT)
        nc.vector.tensor_copy(ones1[:], ones1f[:])

        # weights: load fp32, round on-chip to the matmul dtype
        w_sb = []
        with tc.tile_pool(name="wraw", bufs=2) as wraw:
            for name, wd in (("wqs", wq_d), ("wks", wk_d), ("wvs", wv_d)):
                raw = wraw.tile([128, ndc, d2], F32, tag="wr", name=f"raw_{name}")
                nc.sync.dma_start(out=raw[:],
                                  in_=wd.rearrange("(dc p) m -> p dc m", p=128))
                t = singles.tile([128, ndc, d2], MDT, tag=name, name=name)
                nc.vector.tensor_copy(t[:], raw[:])
                w_sb.append(t)
            raw = wraw.tile([d2, D], F32, tag="wr", name="raw_wo")
            nc.sync.dma_start(out=raw[:], in_=wo_d)
            wo_sb = singles.tile([d2, D], MDT)
            nc.vector.tensor_copy(wo_sb[:], raw[:])

        b_sb = [None, None, None]
        if with_qkv_bias:
            for i, bd in enumerate((bq_d, bk_d, bv_d)):
                t = singles.tile([d2, 1], F32, tag=f"b{i}", name=f"b{i}")
                nc.sync.dma_start(out=t[:], in_=bd)
                b_sb[i] = t
        bo_sb = None
        if with_o_bias:
            bo_sb = singles.tile([128, D], F32)
            nc.gpsimd.dma_start(out=bo_sb[:], in_=bo_d.partition_broadcast(128))

        q2t = singles.tile([128, T], MDT, tag="q2t")
        k2t = singles.tile([128, T], MDT, tag="k2t")
        out2t = singles.tile([128, T], MDT, tag="out2t")
        # vnat[:, u, kt, 0:64] = V rows (k on partitions); col 64 = ones
        vnat = singles.tile([128, nb * hc, KT, dh + 1], MDT, tag="vnat")
        onesc = singles.tile([128, nb * hc, KT, 1], F32)
        nc.vector.memset(onesc[:], 1.0)
        nc.vector.tensor_copy(vnat[:, :, :, dh:dh + 1], onesc[:])

        # ---------- emission helpers ----------
        def p12_chunk(n):
            """Load xT chunk n (512 tokens), project to q/k/v, transpose V."""
            if dma_f32r:
                xt_n = xtpool.tile([128, ndc, 512], MDT, tag="xt",
                                   name=f"xt{n}")
                nc.sync.dma_start(out=xt_n[:],
                                  in_=xt_view[:, :, n * 512:(n + 1) * 512])
            else:
                xr = xtpool.tile([128, ndc, 512], F32, tag="xr", name=f"xr{n}")
                nc.sync.dma_start(out=xr[:],
                                  in_=xt_view[:, :, n * 512:(n + 1) * 512])
                xt_n = xtpool.tile([128, ndc, 512], MDT, tag="xt",
                                   name=f"xt{n}")
                nc.vector.tensor_copy(xt_n[:], xr[:])
            for p in range(3):
                ps = pspool.tile([128, 512], F32, tag="ps", name=f"pj{n}_{p}")
                for dc in range(ndc):
                    nc.tensor.matmul(ps[:], w_sb[p][:, dc, :], xt_n[:, dc, :],
                                     start=(dc == 0), stop=(dc == ndc - 1))
                if p < 2:
                    dst = (q2t, k2t)[p]
                    if with_qkv_bias:
                        nc.vector.tensor_scalar_add(
                            dst[:, n * 512:(n + 1) * 512], ps[:], b_sb[p][:])
                    else:
                        nc.vector.tensor_copy(dst[:, n * 512:(n + 1) * 512],
                                              ps[:])
                else:
                    v2_n = v2pool.tile([128, 512], F32, tag="v2")
                    if with_qkv_bias:
                        nc.vector.tensor_scalar_add(v2_n[:], ps[:], b_sb[2][:])
                    else:
                        nc.vector.tensor_copy(v2_n[:], ps[:])
                    b = (n * 512) // S
                    kt0 = (n * 512 % S) // 128
                    pv = pspool.tile([128, 4, 128], F32, tag="ps",
                                     name=f"pv{n}")
                    for sub in range(4):
                        nc.tensor.transpose(
                            pv[:, sub, :], v2_n[:, sub * 128:(sub + 1) * 128],
                            ident[:])
                    for h in range(hc):
                        nc.vector.tensor_copy(
                            vnat[:, b * hc + h, kt0:kt0 + 4, 0:dh],
                            pv[:, :, h * dh:(h + 1) * dh])

        def p4_tile(i):
            """Output-projection tile i (i indexes (m, j) pairs)."""
            m, j = divmod(i, D // NJ)
            py = pspool.tile([128, NJ], F32, tag="ps", name=f"py{i}")
            nc.tensor.matmul(py[:], out2t[:, m * 128:(m + 1) * 128],
                             wo_sb[:, j * NJ:(j + 1) * NJ],
                             start=True, stop=True)
            yt = ysb.tile([128, NJ], F32, tag="yt")
            if with_o_bias:
                nc.vector.tensor_add(yt[:], py[:], bo_sb[:, j * NJ:(j + 1) * NJ])
            else:
                nc.vector.tensor_copy(yt[:], py[:])
            nc.sync.dma_start(out=y_d[m * 128:(m + 1) * 128,
                                      j * NJ:(j + 1) * NJ], in_=yt[:])

        def finish_qc(u, po_sb, qc):
            """Deferred softmax normalization for unit u, q-chunk qc."""
            b, h = divmod(u, hc)
            hp = slice(h * dh, (h + 1) * dh)
            qcol = slice(b * S + qc * 512, b * S + (qc + 1) * 512)
            rc = rcpool.tile([1, 512], MDT, tag="rc")
            with nc.allow_low_precision(reason="softmax denom"):
                nc.vector.reciprocal(rc[:], po_sb[dh:dh + 1, qc, :])
            rcp = pspool.tile([dh, 512], F32, tag="ps", name=f"rcp{u}_{qc}")
            nc.tensor.matmul(rcp[:], ones1[:], rc[:], start=True, stop=True)
            nc.vector.tensor_mul(out2t[hp, qcol], po_sb[0:dh, qc, :], rcp[:])

        # ---------- interleaved emission ----------
        units = list(range(nb * hc))
        # filler streams: batch-1.. projections ride along with batch-0
        # attention; early batches' output projection rides along with the
        # last batch's attention.
        later_chunks = [n for n in range(cpb, nb * cpb)]
        p4_order = list(range((T // 128) * (D // NJ)))
        p4_per_batch = len(p4_order) // nb

        # assign fillers to units: units of batch 0 get the projection
        # chunks of later batches; units of batch >0 get p4 tiles of
        # earlier batches (which are finished by then).
        fillers = {u: [] for u in units}
        nch_units = [u for u in units if u < hc] or units
        for idx, n in enumerate(later_chunks):
            fillers[nch_units[idx * len(nch_units) // max(1, len(later_chunks))]
                    ].append(("chunk", n))
        p4_tail = []
        for i in p4_order:
            b_of_tile = (i // (D // NJ)) * 128 // S
            # can only ride with units of a LATER batch
            host_units = [u for u in units if u // hc > b_of_tile]
            if host_units:
                fillers[host_units[(i % p4_per_batch) * len(host_units)
                                   // p4_per_batch]].append(("p4", i))
            else:
                p4_tail.append(i)

        # initial projections for batch 0
        for n in range(cpb):
            p12_chunk(n)

        prev_finish = None  # (u, po_sb) awaiting normalization
        for u in units:
            b, h = divmod(u, hc)
            hp = slice(h * dh, (h + 1) * dh)
            halves = [list(range(qh * 2, min(qh * 2 + 2, QC)))
                      for qh in range((QC + 1) // 2)]
            nsteps = len(halves) * KT
            todo = list(fillers[u])
            nfill = len(todo)
            fin_items = ([] if prev_finish is None else
                         [(prev_finish[0], prev_finish[1], qc)
                          for qc in range(QC)])
            k0 = len(fin_items)
            po_sb = posb.tile([dh + 1, QC, 512], F32, tag="posb",
                              name=f"posb{u}")

            step = 0
            for qcs in halves:
                g = len(qcs)
                po = [psO.tile([dh + 1, 512], F32, tag="po",
                               name=f"po{u}_{qc}") for qc in qcs]

                def issue_po(kt, exs):
                    for i in range(g):
                        nc.tensor.matmul(po[i][:], vnat[:, u, kt, :], exs[i],
                                         start=(kt == 0), stop=(kt == KT - 1))

                prev_exs = None
                for kt in range(KT):
                    kcol = slice(b * S + kt * 128, b * S + (kt + 1) * 128)
                    sc2 = pspool.tile([128, g, 512], F32, tag="sc",
                                      name=f"sc{u}_{qcs[0]}_{kt}")
                    for i, qc in enumerate(qcs):
                        qcol = slice(b * S + qc * 512, b * S + (qc + 1) * 512)
                        nc.tensor.matmul(sc2[:, i, :], k2t[hp, kcol],
                                         q2t[hp, qcol], start=True, stop=True)
                    ex2 = epool.tile([128, g, 512], MDT, tag="ex",
                                     name=f"ex{u}_{qcs[0]}_{kt}")
                    nc.scalar.activation(ex2[:], sc2[:], AF.Exp, scale=scale)
                    exs = [ex2[:, i, :] for i in range(g)]
                    if kt > 0:
                        issue_po(kt - 1, prev_exs)
                    prev_exs = exs

                    # deferred finish of the previous unit, then fillers
                    if step < k0:
                        fu, fpo, fqc = fin_items[step]
                        finish_qc(fu, fpo, fqc)
                    elif nfill:
                        lo = nfill * (step - k0) // (nsteps - k0)
                        hi = nfill * (step - k0 + 1) // (nsteps - k0)
                        for kind, arg in todo[lo:hi]:
                            if kind == "chunk":
                                p12_chunk(arg)
                            else:
                                p4_tile(arg)
                    step += 1
                issue_po(KT - 1, prev_exs)
                # drain po -> SBUF so PSUM banks free quickly; the
                # normalization is deferred into the next unit's stream
                for i, qc in enumerate(qcs):
                    nc.vector.tensor_copy(po_sb[:, qc, :], po[i][:])
            prev_finish = (u, po_sb)

        for qc in range(QC):
            finish_qc(prev_finish[0], prev_finish[1], qc)
        for i in p4_tail:
            p4_tile(i)

    nc.compile()
    return nc


_PROGRAM_CACHE = {}


def _get_program(key):
    if key not in _PROGRAM_CACHE:
        with_qkv_bias, with_o_bias = key
        _PROGRAM_CACHE[key] = build_program(
            with_qkv_bias=with_qkv_bias, with_o_bias=with_o_bias)
    return _PROGRAM_CACHE[key]


def _round_tf32(a):
    """Round fp32 to tf32 (10-bit mantissa), round-to-nearest-even."""
    u = a.view(np.uint32)
    r = (u + 0xFFF + ((u >> 13) & 1)) & np.uint32(0xFFFFE000)
    return r.view(np.float32)


def make_in_maps(x, wq, bq, wk, bk, wv, bv, wo, bo, with_qkv_bias, with_o_bias,
                 n_cores=N_CORES, hc=HEADS_PER_CORE, dh=DEPTH):
    d2 = hc * dh
    xt = _round_tf32(np.ascontiguousarray(x.T))
    in_maps = []
    for c in range(n_cores):
        cs = slice(c * d2, (c + 1) * d2)
        m = {"xt": xt,
             "wq": np.ascontiguousarray(wq[:, cs]),
             "wk": np.ascontiguousarray(wk[:, cs]),
             "wv": np.ascontiguousarray(wv[:, cs]),
             "wo": np.ascontiguousarray(wo[cs, :])}
        if with_qkv_bias:
            m["bq"] = np.ascontiguousarray(bq[cs].reshape(d2, 1))
            m["bk"] = np.ascontiguousarray(bk[cs].reshape(d2, 1))
            m["bv"] = np.ascontiguousarray(bv[cs].reshape(d2, 1))
        if with_o_bias:
            m["bo"] = (bo.reshape(1, -1).astype(np.float32) if c == 0
                       else np.zeros((1, bo.shape[-1]), np.float32))
        in_maps.append(m)
    return in_maps


def kernel(inputs, wq, bq, wk, bk, wv, bv, wo, bo):
    x = np.ascontiguousarray(np.asarray(inputs, np.float32)
                             .reshape(B_FULL * S_FULL, D_MODEL))
    wq, wk, wv, wo = (np.asarray(a, np.float32) for a in (wq, wk, wv, wo))
    bq, bk, bv, bo = (np.asarray(a, np.float32) for a in (bq, bk, bv, bo))

    with_qkv_bias = bool(np.any(bq) or np.any(bk) or np.any(bv))
    with_o_bias = bool(np.any(bo))
    nc = _get_program((with_qkv_bias, with_o_bias))

    in_maps = make_in_maps(x, wq, bq, wk, bk, wv, bv, wo, bo,
                           with_qkv_bias, with_o_bias)
    res = run_bass_kernel_spmd(nc, in_maps, list(range(N_CORES))).results
    y = np.zeros((B_FULL * S_FULL, D_MODEL), np.float64)
    for c in range(N_CORES):
        y += res[c]["y"]
    return y.astype(np.float32).reshape(B_FULL, S_FULL, D_MODEL)


# revision 25
# speedup vs baseline: 1.9892x; 1.2897x over previous
"""Multi-head self-attention (B=2, S=2048, D=1024, H=16) on 8 TRN2 NeuronCores.

Sharding: head-parallel — 2 heads per core. Each core computes Q/K/V
projections for its 2 heads over all B*S tokens, full (non-causal)
softmax attention for its 4 (batch, head) units, and a partial output
projection y_c = sum_h out_h @ wo[h]. Host sums the 8 partial outputs.
The host also pre-transposes x to xT (pure layout prep) so the device
reads the contraction dim on partitions directly.

Device dataflow (head-dim on partitions):
  q2t/k2t/v2t [128=2*64, T] = w[:,2heads]^T @ xT      (PSUM accum over D)
  v2t --PE transpose--> vnat [k, d] (+ ones column -> denominator row)
  scoresT[k, q] = K Q^T  (contract d=64), exp on ACT (scale=1/8 folded in)
  poT[d+1, q] += vnat^T @ exp  accumulated over k tiles  (PSUM)
  out2t[:, q] = poT[0:64] * (1/poT[64]) broadcast (K=1 matmul + DVE mul)
  y[s, n] = out2t[:, s-tile]^T @ wo2   (contract 128 = 2 heads * 64)

Matmuls run as float32r (tfloat32 datapath, 1 cycle/row); every operand
tile is produced with an fp32r-rounding instruction as the BIR verifier
requires. PSUM accumulation stays fp32.

Emission is phase-interleaved so the PE always has independent filler
work while the ACT engine grinds through the exps: projections for
batch 1 ride along with batch 0's attention, and batch 0's output
projection rides along with batch 1's attention.
"""

import numpy as np
from contextlib import ExitStack

import concourse.bass as bass
import concourse.tile as tile
from concourse import bacc, mybir
from concourse.bass_utils import run_bass_kernel_spmd
from concourse.masks import make_identity

F32 = mybir.dt.float32
F32R = mybir.dt.float32r
AF = mybir.ActivationFunctionType

N_CORES = 8
D_MODEL = 1024
NUM_HEADS = 16
DEPTH = 64
HEADS_PER_CORE = NUM_HEADS // N_CORES  # 2
B_FULL = 2
S_FULL = 2048


def build_program(T=4096, D=1024, S=2048, dh=64, hc=2, with_qkv_bias=False,
                  with_o_bias=False, use_f32r=True, dma_f32r=True):
    """Build the SPMD Bass program for one core (hc heads).

    T: total tokens (B*S); D: model dim; S: seq len per batch; dh: head depth;
    hc: heads per core. Requires hc*dh == 128, D % 128 == 0, S % 512 == 0,
    T % S == 0. dma_f32r: DMA x directly into fp32r tiles (if the verifier
    allows DMA producers); else DMA to fp32 and round via DVE copy.
    """
    d2 = hc * dh
    assert d2 == 128 and D % 128 == 0 and S % 512 == 0 and T % S == 0
    nb = T // S            # batches
    ndc = D // 128         # D chunks (contraction tiles)
    cpb = S // 512         # 512-token chunks per batch
    KT = S // 128          # k tiles per (b,h) unit
    QC = S // 512          # 512-wide q chunks per batch
    NJ = min(512, D)
    scale = 1.0 / float(np.sqrt(dh))
    MDT = F32R if use_f32r else F32

    nc = bacc.Bacc("TRN2", target_bir_lowering=False, debug=False,
                   num_devices=N_CORES)

    xt_d = nc.dram_tensor("xt", [D, T], F32R if dma_f32r else F32,
                          kind="ExternalInput").ap()
    wq_d = nc.dram_tensor("wq", [D, d2], F32, kind="ExternalInput").ap()
    wk_d = nc.dram_tensor("wk", [D, d2], F32, kind="ExternalInput").ap()
    wv_d = nc.dram_tensor("wv", [D, d2], F32, kind="ExternalInput").ap()
    wo_d = nc.dram_tensor("wo", [d2, D], F32, kind="ExternalInput").ap()
    if with_qkv_bias:
        bq_d = nc.dram_tensor("bq", [d2, 1], F32, kind="ExternalInput").ap()
        bk_d = nc.dram_tensor("bk", [d2, 1], F32, kind="ExternalInput").ap()
        bv_d = nc.dram_tensor("bv", [d2, 1], F32, kind="ExternalInput").ap()
    if with_o_bias:
        bo_d = nc.dram_tensor("bo", [1, D], F32, kind="ExternalInput").ap()
    y_d = nc.dram_tensor("y", [T, D], F32, kind="ExternalOutput").ap()

    xt_view = xt_d.rearrange("(dc p) t -> p dc t", p=128)

    with tile.TileContext(nc) as tc, ExitStack() as ctx:
        singles = ctx.enter_context(tc.tile_pool(name="singles", bufs=1))
        xtpool = ctx.enter_context(tc.tile_pool(name="xtpool", bufs=3))
        v2pool = ctx.enter_context(tc.tile_pool(name="v2pool", bufs=2))
        epool = ctx.enter_context(tc.tile_pool(name="epool", bufs=4))
        ysb = ctx.enter_context(tc.tile_pool(name="ysb", bufs=3))
        rcpool = ctx.enter_context(tc.tile_pool(name="rcpool", bufs=2))
        # PSUM budget (8 banks): sc 2x[128,2,512]=4, ps 2x[128,512]=2,
        # po 2x[65,512]=2
        pspool = ctx.enter_context(tc.tile_pool(name="ps", bufs=2, space="PSUM"))
        psO = ctx.enter_context(tc.tile_pool(name="psO", bufs=2, space="PSUM"))
        posb = ctx.enter_context(tc.tile_pool(name="posb", bufs=2))

        ident = singles.tile([128, 128], F32)
        make_identity(nc, ident[:])
        ones1f = singles.tile([1, dh], F32)
        nc.vector.memset(ones1f[:], 1.0)
        ones1 = singles.tile([1, dh], MDT)
        nc.vector.tensor_copy(ones1[:], ones1f[:])

        # weights: load fp32, round on-chip to the matmul dtype
        w_sb = []
        with tc.tile_pool(name="wraw", bufs=2) as wraw:
            for name, wd in (("wqs", wq_d), ("wks", wk_d), ("wvs", wv_d)):
                raw = wraw.tile([128, ndc, d2], F32, tag="wr", name=f"raw_{name}")
                nc.sync.dma_start(out=raw[:],
                                  in_=wd.rearrange("(dc p) m -> p dc m", p=128))
                t = singles.tile([128, ndc, d2], MDT, tag=name, name=name)
                nc.vector.tensor_copy(t[:], raw[:])
                w_sb.append(t)
            raw = wraw.tile([d2, D], F32, tag="wr", name="raw_wo")
            nc.sync.dma_start(out=raw[:], in_=wo_d)
            wo_sb = singles.tile([d2, D], MDT)
            nc.vector.tensor_copy(wo_sb[:], raw[:])

        b_sb = [None, None, None]
        if with_qkv_bias:
            for i, bd in enumerate((bq_d, bk_d, bv_d)):
                t = singles.tile([d2, 1], F32, tag=f"b{i}", name=f"b{i}")
                nc.sync.dma_start(out=t[:], in_=bd)
                b_sb[i] = t
        bo_sb = None
        if with_o_bias:
            bo_sb = singles.tile([128, D], F32)
            nc.gpsimd.dma_start(out=bo_sb[:], in_=bo_d.partition_broadcast(128))

        # Q stored zero-padded per head: q2tz[h] has head h's Q^T on its own
        # 64 partitions and ZEROS on the other 64. The score matmul then runs
        # with the full [128,128] two-head K tile as stationary (K=128
        # contraction) — fp32r at K=64 is half-rate, K=128 is full-rate —
        # and the zero rows cancel the other head's contribution.
        q2tz = [singles.tile([128, T], MDT, tag=f"q2tz{h}", name=f"q2tz{h}")
                for h in range(hc)]
        if dh < 128:
            for h in range(hc):
                zrows = (slice(dh, 128) if h == 0 else slice(0, h * dh))
                nc.vector.memset(q2tz[h][zrows, :].bitcast(F32), 0.0)
        k2t = singles.tile([128, T], MDT, tag="k2t")
        out2t = singles.tile([128, T], MDT, tag="out2t")
        # vnat[:, u, kt, 0:64] = V rows (k on partitions); col 64 = ones
        vnat = singles.tile([128, nb * hc, KT, dh + 1], MDT, tag="vnat")
        onesc = singles.tile([128, nb * hc, KT, 1], F32)
        nc.vector.memset(onesc[:], 1.0)
        nc.vector.tensor_copy(vnat[:, :, :, dh:dh + 1], onesc[:])

        # ---------- emission helpers ----------
        def p12_chunk(n):
            """Load xT chunk n (512 tokens), project to q/k/v, transpose V."""
            if dma_f32r:
                xt_n = xtpool.tile([128, ndc, 512], MDT, tag="xt",
                                   name=f"xt{n}")
                nc.sync.dma_start(out=xt_n[:],
                                  in_=xt_view[:, :, n * 512:(n + 1) * 512])
            else:
                xr = xtpool.tile([128, ndc, 512], F32, tag="xr", name=f"xr{n}")
                nc.sync.dma_start(out=xr[:],
                                  in_=xt_view[:, :, n * 512:(n + 1) * 512])
                xt_n = xtpool.tile([128, ndc, 512], MDT, tag="xt",
                                   name=f"xt{n}")
                nc.vector.tensor_copy(xt_n[:], xr[:])
            for p in range(3):
                ps = pspool.tile([128, 512], F32, tag="ps", name=f"pj{n}_{p}")
                for dc in range(ndc):
                    nc.tensor.matmul(ps[:], w_sb[p][:, dc, :], xt_n[:, dc, :],
                                     start=(dc == 0), stop=(dc == ndc - 1))
                if p == 0:
                    ncol = slice(n * 512, (n + 1) * 512)
                    for h in range(hc):
                        hp_ = slice(h * dh, (h + 1) * dh)
                        if with_qkv_bias:
                            nc.vector.tensor_scalar_add(
                                q2tz[h][hp_, ncol], ps[hp_, :], b_sb[0][hp_, :])
                        else:
                            nc.vector.tensor_copy(q2tz[h][hp_, ncol],
                                                  ps[hp_, :])
                elif p == 1:
                    if with_qkv_bias:
                        nc.vector.tensor_scalar_add(
                            k2t[:, n * 512:(n + 1) * 512], ps[:], b_sb[1][:])
                    else:
                        nc.vector.tensor_copy(k2t[:, n * 512:(n + 1) * 512],
                                              ps[:])
                else:
                    v2_n = v2pool.tile([128, 512], F32, tag="v2")
                    if with_qkv_bias:
                        nc.vector.tensor_scalar_add(v2_n[:], ps[:], b_sb[2][:])
                    else:
                        nc.vector.tensor_copy(v2_n[:], ps[:])
                    b = (n * 512) // S
                    kt0 = (n * 512 % S) // 128
                    pv = pspool.tile([128, 4, 128], F32, tag="ps",
                                     name=f"pv{n}")
                    for sub in range(4):
                        nc.tensor.transpose(
                            pv[:, sub, :], v2_n[:, sub * 128:(sub + 1) * 128],
                            ident[:])
                    for h in range(hc):
                        nc.vector.tensor_copy(
                            vnat[:, b * hc + h, kt0:kt0 + 4, 0:dh],
                            pv[:, :, h * dh:(h + 1) * dh])

        def p4_tile(i):
            """Output-projection tile i (i indexes (m, j) pairs)."""
            m, j = divmod(i, D // NJ)
            py = pspool.tile([128, NJ], F32, tag="ps", name=f"py{i}")
            nc.tensor.matmul(py[:], out2t[:, m * 128:(m + 1) * 128],
                             wo_sb[:, j * NJ:(j + 1) * NJ],
                             start=True, stop=True)
            yt = ysb.tile([128, NJ], F32, tag="yt")
            if with_o_bias:
                nc.vector.tensor_add(yt[:], py[:], bo_sb[:, j * NJ:(j + 1) * NJ])
            else:
                nc.vector.tensor_copy(yt[:], py[:])
            nc.sync.dma_start(out=y_d[m * 128:(m + 1) * 128,
                                      j * NJ:(j + 1) * NJ], in_=yt[:])

        def finish_qc(u, po_sb, qc):
            """Deferred softmax normalization for unit u, q-chunk qc."""
            b, h = divmod(u, hc)
            hp = slice(h * dh, (h + 1) * dh)
            qcol = slice(b * S + qc * 512, b * S + (qc + 1) * 512)
            rc = rcpool.tile([1, 512], MDT, tag="rc")
            with nc.allow_low_precision(reason="softmax denom"):
                nc.vector.reciprocal(rc[:], po_sb[dh:dh + 1, qc, :])
            rcp = pspool.tile([dh, 512], F32, tag="ps", name=f"rcp{u}_{qc}")
            nc.tensor.matmul(rcp[:], ones1[:], rc[:], start=True, stop=True)
            nc.vector.tensor_mul(out2t[hp, qcol], po_sb[0:dh, qc, :], rcp[:])

        # ---------- interleaved emission ----------
        units = list(range(nb * hc))
        # filler streams: batch-1.. projections ride along with batch-0
        # attention; early batches' output projection rides along with the
        # last batch's attention.
        later_chunks = [n for n in range(cpb, nb * cpb)]
        p4_order = list(range((T // 128) * (D // NJ)))
        p4_per_batch = len(p4_order) // nb

        # assign fillers to units: units of batch 0 get the projection
        # chunks of later batches; units of batch >0 get p4 tiles of
        # earlier batches (which are finished by then).
        fillers = {u: [] for u in units}
        nch_units = [u for u in units if u < hc] or units
        for idx, n in enumerate(later_chunks):
            fillers[nch_units[idx * len(nch_units) // max(1, len(later_chunks))]
                    ].append(("chunk", n))
        p4_tail = []
        for i in p4_order:
            b_of_tile = (i // (D // NJ)) * 128 // S
            # can only ride with units of a LATER batch
            host_units = [u for u in units if u // hc > b_of_tile]
            if host_units:
                fillers[host_units[(i % p4_per_batch) * len(host_units)
                                   // p4_per_batch]].append(("p4", i))
            else:
                p4_tail.append(i)

        # initial projections for batch 0
        for n in range(cpb):
            p12_chunk(n)

        prev_finish = None  # (u, po_sb) awaiting normalization
        for u in units:
            b, h = divmod(u, hc)
            hp = slice(h * dh, (h + 1) * dh)
            halves = [list(range(qh * 2, min(qh * 2 + 2, QC)))
                      for qh in range((QC + 1) // 2)]
            nsteps = len(halves) * KT
            todo = list(fillers[u])
            nfill = len(todo)
            fin_items = ([] if prev_finish is None else
                         [(prev_finish[0], prev_finish[1], qc)
                          for qc in range(QC)])
            k0 = len(fin_items)
            po_sb = posb.tile([dh + 1, QC, 512], F32, tag="posb",
                              name=f"posb{u}")

            step = 0
            for qcs in halves:
                g = len(qcs)
                po = [psO.tile([dh + 1, 512], F32, tag="po",
                               name=f"po{u}_{qc}") for qc in qcs]

                def issue_po(kt, exs):
                    for i in range(g):
                        nc.tensor.matmul(po[i][:], vnat[:, u, kt, :], exs[i],
                                         start=(kt == 0), stop=(kt == KT - 1))

                prev_exs = None
                for kt in range(KT):
                    kcol = slice(b * S + kt * 128, b * S + (kt + 1) * 128)
                    sc2 = pspool.tile([128, g, 512], F32, tag="sc",
                                      name=f"sc{u}_{qcs[0]}_{kt}")
                    for i, qc in enumerate(qcs):
                        qcol = slice(b * S + qc * 512, b * S + (qc + 1) * 512)
                        nc.tensor.matmul(sc2[:, i, :], k2t[:, kcol],
                                         q2tz[h][:, qcol],
                                         start=True, stop=True)
                    ex2 = epool.tile([128, g, 512], MDT, tag="ex",
                                     name=f"ex{u}_{qcs[0]}_{kt}")
                    nc.scalar.activation(ex2[:], sc2[:], AF.Exp, scale=scale)
                    exs = [ex2[:, i, :] for i in range(g)]
                    if kt > 0:
                        issue_po(kt - 1, prev_exs)
                    prev_exs = exs

                    # deferred finish of the previous unit, then fillers
                    if step < k0:
                        fu, fpo, fqc = fin_items[step]
                        finish_qc(fu, fpo, fqc)
                    elif nfill:
                        lo = nfill * (step - k0) // (nsteps - k0)
                        hi = nfill * (step - k0 + 1) // (nsteps - k0)
                        for kind, arg in todo[lo:hi]:
                            if kind == "chunk":
                                p12_chunk(arg)
                            else:
                                p4_tile(arg)
                    step += 1
                issue_po(KT - 1, prev_exs)
                # drain po -> SBUF so PSUM banks free quickly; the
                # normalization is deferred into the next unit's stream
                for i, qc in enumerate(qcs):
                    nc.vector.tensor_copy(po_sb[:, qc, :], po[i][:])
            prev_finish = (u, po_sb)

        for qc in range(QC):
            finish_qc(prev_finish[0], prev_finish[1], qc)
        for i in p4_tail:
            p4_tile(i)

    nc.compile()
    return nc


_PROGRAM_CACHE = {}


def _get_program(key):
    if key not in _PROGRAM_CACHE:
        with_qkv_bias, with_o_bias = key
        _PROGRAM_CACHE[key] = build_program(
            with_qkv_bias=with_qkv_bias, with_o_bias=with_o_bias)
    return _PROGRAM_CACHE[key]


def _round_tf32(a):
    """Round fp32 to tf32 (10-bit mantissa), round-to-nearest-even."""
    u = a.view(np.uint32)
    r = (u + 0xFFF + ((u >> 13) & 1)) & np.uint32(0xFFFFE000)
    return r.view(np.float32)


def make_in_maps(x, wq, bq, wk, bk, wv, bv, wo, bo, with_qkv_bias, with_o_bias,
                 n_cores=N_CORES, hc=HEADS_PER_CORE, dh=DEPTH):
    d2 = hc * dh
    xt = _round_tf32(np.ascontiguousarray(x.T))
    in_maps = []
    for c in range(n_cores):
        cs = slice(c * d2, (c + 1) * d2)
        m = {"xt": xt,
             "wq": np.ascontiguousarray(wq[:, cs]),
             "wk": np.ascontiguousarray(wk[:, cs]),
             "wv": np.ascontiguousarray(wv[:, cs]),
             "wo": np.ascontiguousarray(wo[cs, :])}
        if with_qkv_bias:
            m["bq"] = np.ascontiguousarray(bq[cs].reshape(d2, 1))
            m["bk"] = np.ascontiguousarray(bk[cs].reshape(d2, 1))
            m["bv"] = np.ascontiguousarray(bv[cs].reshape(d2, 1))
        if with_o_bias:
            m["bo"] = (bo.reshape(1, -1).astype(np.float32) if c == 0
                       else np.zeros((1, bo.shape[-1]), np.float32))
        in_maps.append(m)
    return in_maps


def kernel(inputs, wq, bq, wk, bk, wv, bv, wo, bo):
    x = np.ascontiguousarray(np.asarray(inputs, np.float32)
                             .reshape(B_FULL * S_FULL, D_MODEL))
    wq, wk, wv, wo = (np.asarray(a, np.float32) for a in (wq, wk, wv, wo))
    bq, bk, bv, bo = (np.asarray(a, np.float32) for a in (bq, bk, bv, bo))

    with_qkv_bias = bool(np.any(bq) or np.any(bk) or np.any(bv))
    with_o_bias = bool(np.any(bo))
    nc = _get_program((with_qkv_bias, with_o_bias))

    in_maps = make_in_maps(x, wq, bq, wk, bk, wv, bv, wo, bo,
                           with_qkv_bias, with_o_bias)
    res = run_bass_kernel_spmd(nc, in_maps, list(range(N_CORES))).results
    y = np.zeros((B_FULL * S_FULL, D_MODEL), np.float64)
    for c in range(N_CORES):
        y += res[c]["y"]
    return y.astype(np.float32).reshape(B_FULL, S_FULL, D_MODEL)
